# revision 17
# baseline (speedup 1.0000x reference)
"""Ensemble-SRN MoE routing kernel for 8 TRN2 NeuronCores.

Strategy: expert-parallel sharding. The 8 experts are axis-aligned octants of
[-1,1]^3 (GRID=(2,2,2)); core e receives exactly the points routed to expert e
(the all-to-all dispatch happens on the host as part of sharding), runs a dense
single-expert 3->64->64->1 ReLU MLP over its (padded) shard, and the host
inverse-permutes the outputs.

v5 design (driven by HW microbenchmarks):
  - The PE array is 16 interleaved 32x32 sub-arrays; small-K matmuls issued to
    DISTINCT row-groups via tile_position=(32g,0) run CONCURRENTLY (measured:
    4x K=8 N=512 serial 1852ns -> row-tiled 283ns). L1 (K=6) uses this: the
    4 pairs of a quad live in row-groups 0..3 of x_all and their four L1
    matmuls run in one ~280ns burst. The S1 stationary holds the w1 block-diag
    replicated at all 4 row offsets.
  - L3 col-tiles via tile_position=(0,32p) (measured 4x K=128 M=16 N=512 in
    ~126ns) - effectively free.
  - Stationary switches measured ~36ns (not the ~200ns v4 assumed), so no
    multi-round layer batching is needed.
  - Evacuation (PSUM->SBUF relu+bias, bf16) is now the wall: DVE 1.13ns/col,
    ACT 1.05ns/col alone; in parallel with separate PSUM banks and separate
    SBUF dst tiles ~1197ns per [128,1024] op. Per quad: each engine does one
    h1 duo-evac + one h2 duo-evac (~2.4us) vs PE ~1.4us.
  - L2 writes back into the SAME psa duo tile its L1 used (freed by the h1
    evac): psa = 3x[128,1024] duo tiles (6 banks) + 2 y banks = 8.
  - L3 uses an M=16 stationary holding [w3;0 | 0;w3] at column offset
    2*(r%8): eight quads accumulate into disjoint row-pairs of one PSUM
    bank (start only at r%8==0), so the y evacuation runs once per 8 quads.
  - PE warmup matmuls + ACT table preload before the main body overlap the
    input DMAs.

Layout: quad r covers pairs 4r..4r+3; pair i at row-group g=i%4, slot s,
point n (pair i covers points i*1024..(i+1)*1024, slot s = n//512):
  x_all[32g + 3s + c, r*512 + n] : coord c of point (pair i=4r+g, slot s, n)
  L1: S1[32g+(0:3), 0:64]=W1' / S1[32g+(3:6), 64:128]=W1' -> psa duo halves
  L2: w2 block-diag                 -> same psa half (after h1 evac)
  L3: w3k (k=r%8)                   -> ypsum[32*g : +16, :] accumulate
  y group flush: ypsum -> y_acc -> yO[p, 2k+s, g*512+n]
"""

import ml_dtypes
import numpy as np

import concourse.bass as bass
import concourse.tile as tile
from concourse import bacc, mybir
from concourse.bass_utils import run_bass_kernel_spmd

F32 = mybir.dt.float32
BF16 = mybir.dt.bfloat16

N_CORES = 8
GRID = (2, 2, 2)
H = 64
F = 512              # points-per-slot per tile (one PSUM-bank free dim, fp32)
PTS_PER_ROUND = 4096
RG = 8               # rounds per y-accumulation group

_PROGRAM_CACHE = {}
LAST_RESULTS = None  # BassKernelResults of the last run (for test harness)
LAST_IN_MAPS = None  # per-core input dicts of the last run (for test harness)
LAST_NC = None       # compiled program of the last run (for test harness)
LAST_P = None        # n_pairs of the last run (for test harness)


def _build_program(n_pairs, loop_n=None, warmup=5, mode="full",
                   evac="alt", order="interleaved", dummies=0, dcols=384):
    """Build the SPMD program for n_pairs pairs (1024 points each); the last
    quad (of 4 pairs / 4096 points) may be partial.

    loop_n (bench only): repeat the whole body loop_n times in a hardware
    For_i so device time can be measured through the noisy axon dispatch
    path by differencing two loop counts."""
    nc = bacc.Bacc(
        "TRN2",
        target_bir_lowering=False,
        debug=False,
        num_devices=N_CORES,
    )
    P = n_pairs
    nr = (P + 3) // 4
    QCOL = nr * F
    n_groups = (nr + RG - 1) // RG
    GCOL = n_groups * F
    # wb: S1 [0:128], w2 [128:256], w3 blocks k=0..7 [256+16k : 272+16k]
    xT = nc.dram_tensor("xT", [24, QCOL], BF16, kind="ExternalInput")
    wb = nc.dram_tensor("wb", [128, 384], BF16, kind="ExternalInput")
    fb = nc.dram_tensor("fb", [128, 2], F32, kind="ExternalInput")
    # y[p, 2k+s, g*512+n] = output of point (r=g*RG+k, tile 2p+s, n)
    yO = nc.dram_tensor("y", [128, GCOL], F32, kind="ExternalOutput")

    RELU = mybir.ActivationFunctionType.Relu
    ADD = mybir.AluOpType.add
    MAX = mybir.AluOpType.max

    def rduos(r):
        return range(2 * r, min((P + 1) // 2, 2 * r + 2))

    def duo_w(d):
        return min(P - 2 * d, 2)

    with tile.TileContext(nc) as tc:
        with (
            tc.tile_pool(name="const", bufs=1) as const,
            tc.tile_pool(name="h1p", bufs=4) as h1pool,
            tc.tile_pool(name="h2p", bufs=4) as h2pool,
            tc.tile_pool(name="psa", bufs=3, space="PSUM") as psa,
            tc.tile_pool(name="psy", bufs=2, space="PSUM") as psy,
        ):
            # --- PE warmup (p-state ramp) + ACT table preload, overlapping
            # the input DMAs ---
            scratch = const.tile([128, F], BF16)
            nc.vector.memset(scratch[:], 0.0)
            dum = const.tile([128, 1], BF16)
            nc.vector.memset(dum[:], 0.0)
            nc.scalar.activation(
                dum[:], dum[:], mybir.ActivationFunctionType.Relu
            )
            yps0 = psy.tile([128, F], F32, tag="psy")
            yps1 = psy.tile([128, F], F32, tag="psy")
            ypsums = [yps0, yps1]
            for _ in range(warmup):
                nc.tensor.matmul(
                    ypsums[0][0:1, :], scratch[:, 0:1], scratch[:],
                    start=True, stop=True, tile_position=(0, 0),
                )
            nc.vector.memset(ypsums[0][:], 0.0)
            nc.vector.memset(ypsums[1][:], 0.0)

            wb_sb = const.tile([128, 384], BF16)
            nc.scalar.dma_start(wb_sb[:], wb.ap())
            fb_sb = const.tile([128, 2], F32)
            nc.sync.dma_start(fb_sb[:], fb.ap())
            x_all = const.tile([128, QCOL], BF16)
            # xT holds only the 24 live rows (4 groups x 6): one contiguous
            # DMA per row group, issue split across the SP and ACT HWDGE
            # queues so the ~625ns per-dma issue overhead parallelizes
            for g in range(4):
                q = nc.sync if g % 2 == 0 else nc.scalar
                q.dma_start(
                    x_all[32 * g : 32 * g + 6, :],
                    xT.ap()[6 * g : 6 * g + 6, :],
                )
            y_acc = const.tile([128, GCOL], F32)

            w2_sb = wb_sb[:, 128:256]
            b1_sb = fb_sb[:, 0:1]
            b2_sb = fb_sb[:, 1:2]

            import contextlib
            loop_cm = (
                tc.For_i(
                    0, loop_n, 1,
                    hint_engines=(
                        mybir.EngineType.PE,
                        mybir.EngineType.DVE,
                        mybir.EngineType.Activation,
                        mybir.EngineType.SP,
                    ),
                )
                if loop_n
                else contextlib.nullcontext()
            )
            with loop_cm:
                st = {}

                def clock_keep(t, n):
                    # value-neutral PE work (accumulate 0 into the active y
                    # bank) to hold the PE p-state at full clock through the
                    # dependency waits before the L2 duos
                    if n <= 0 or not (0 <= t - 1 < nr):
                        return
                    bank = (max(0, t - 2) // RG) % 2
                    for _ in range(n):
                        nc.tensor.matmul(
                            ypsums[bank][:, 0:dcols],
                            w2_sb,
                            scratch[:, 0:dcols],
                            start=False, stop=False,
                            tile_position=(0, 0),
                            skip_group_check=True,
                        )

                def l1_duo(r, j):
                    # 2 row-tiled L1 matmuls (concurrent on HW); eh1 per duo
                    if not (0 <= r < nr):
                        return
                    for d in rduos(r):
                        if d != 2 * r + j:
                            continue
                        w = duo_w(d)
                        a = psa.tile([128, 2 * F], F32, tag="psa")
                        st[("a", d)] = a
                        for half in range(w):
                            i = 2 * d + half
                            g = i % 4
                            nc.tensor.matmul(
                                a[:, F * half : F * half + F],
                                wb_sb[32 * g : 32 * g + 6, 0:128],
                                x_all[32 * g : 32 * g + 6,
                                      r * F : (r + 1) * F],
                                start=True, stop=True,
                                tile_position=(32 * g, 0),
                            )
                        h1 = h1pool.tile([128, w * F], BF16)
                        st[("h1", d)] = h1
                        if evac == "half" and w == 2:
                            nc.vector.tensor_scalar(
                                h1[:, 0:F], a[:, 0:F], b1_sb, 0.0, ADD, MAX
                            )
                            nc.scalar.activation(
                                h1[:, F : 2 * F], a[:, F : 2 * F], RELU,
                                bias=b1_sb
                            )
                        elif (d % 2 == 0) if evac == "alt" else (
                                evac == "half" or True):
                            nc.vector.tensor_scalar(
                                h1[:], a[:, 0 : w * F], b1_sb, 0.0, ADD, MAX
                            )
                        else:
                            nc.scalar.activation(
                                h1[:], a[:, 0 : w * F], RELU, bias=b1_sb
                            )

                def l2_duo(r, j):
                    # 2 L2 matmuls, shared stationary, writing back into the
                    # (evacuated) psa duo tiles; eh2 per duo
                    if not (0 <= r < nr):
                        return
                    for d in rduos(r):
                        if d != 2 * r + j:
                            continue
                        w = duo_w(d)
                        a = st.pop(("a", d))
                        h1 = st.pop(("h1", d))
                        for half in range(w):
                            nc.tensor.matmul(
                                a[:, F * half : F * half + F],
                                w2_sb,
                                h1[:, F * half : F * half + F],
                                start=True, stop=True,
                                tile_position=(0, 0),
                            )
                        h2 = h2pool.tile([128, w * F], BF16)
                        st[("h2", d)] = h2
                        if evac == "half" and w == 2:
                            nc.vector.tensor_scalar(
                                h2[:, 0:F], a[:, 0:F], b2_sb, 0.0, ADD, MAX
                            )
                            nc.scalar.activation(
                                h2[:, F : 2 * F], a[:, F : 2 * F], RELU,
                                bias=b2_sb
                            )
                        elif (d % 2 == 1) if evac == "alt" else (
                                evac != "half" and False):
                            nc.vector.tensor_scalar(
                                h2[:], a[:, 0 : w * F], b2_sb, 0.0, ADD, MAX
                            )
                        else:
                            nc.scalar.activation(
                                h2[:], a[:, 0 : w * F], RELU, bias=b2_sb
                            )

                def l3_batch(r):
                    # 4 L3 matmuls, shared w3k, col-tiled (concurrent),
                    # accumulating into the group's y bank
                    if not (0 <= r < nr):
                        return
                    g, k = divmod(r, RG)
                    yp = ypsums[g % 2]
                    w3k = wb_sb[:, 256 + 16 * k : 272 + 16 * k]
                    for d in rduos(r):
                        w = duo_w(d)
                        h2 = st.pop(("h2", d))
                        for half in range(w):
                            i = 2 * d + half
                            p = i % 4
                            # last quad of group g still holding strip p
                            lr = min(min(nr, (g + 1) * RG) - 1,
                                     (P - 1 - p) // 4)
                            nc.tensor.matmul(
                                yp[32 * p : 32 * p + 16, :],
                                w3k,
                                h2[:, F * half : F * half + F],
                                start=(k == 0), stop=(r == lr),
                                tile_position=(0, 32 * p),
                                skip_group_check=True,
                            )

                def yflush(r):
                    # after l3_batch of the last quad of group g
                    if not (0 <= r < nr):
                        return
                    g = r // RG
                    if r != min(nr, (g + 1) * RG) - 1:
                        return
                    yp = ypsums[g % 2]
                    dst = y_acc[:, g * F : (g + 1) * F]
                    if g % 2 == 0:
                        nc.vector.tensor_scalar(dst, yp[:], 0.0, None, ADD)
                    else:
                        nc.scalar.activation(
                            dst, yp[:], mybir.ActivationFunctionType.Identity
                        )
                    nc.sync.dma_start(
                        yO.ap()[:, g * F : (g + 1) * F],
                        y_acc[:, g * F : (g + 1) * F],
                    )

                # Per-duo interleave [L2(t-1,d0), L1(t,d0), L2(t-1,d1),
                # L1(t,d1), L3(t-2)]: with h1 evacs on DVE and h2 on ACT,
                # every cross-engine dependency then lands on an op that is
                # early in the producer engine's stream, so the steady-state
                # period is evac-busy-bound (~2.4us/quad) instead of
                # latency-bound.
                for t in range(0, nr + 3):
                    if order == "interleaved":
                        clock_keep(t, dummies)
                        l2_duo(t - 1, 0)
                        l1_duo(t, 0)
                        clock_keep(t, dummies)
                        l2_duo(t - 1, 1)
                        l1_duo(t, 1)
                    else:
                        clock_keep(t, dummies)
                        l2_duo(t - 1, 0)
                        l2_duo(t - 1, 1)
                        l1_duo(t, 0)
                        l1_duo(t, 1)
                    l3_batch(t - 2)
                    yflush(t - 2)

    nc.compile()
    return nc


def kernel(x, extents_min, extents_max, W1, b1, W2, b2, W3, b3):
    global LAST_RESULTS
    x = np.ascontiguousarray(np.asarray(x, dtype=np.float32))
    extents_min = np.asarray(extents_min, dtype=np.float32)
    extents_max = np.asarray(extents_max, dtype=np.float32)
    W1 = np.asarray(W1, dtype=np.float32)
    b1 = np.asarray(b1, dtype=np.float32)
    W2 = np.asarray(W2, dtype=np.float32)
    b2 = np.asarray(b2, dtype=np.float32)
    W3 = np.asarray(W3, dtype=np.float32)
    b3 = np.asarray(b3, dtype=np.float32)

    n_pts = x.shape[0]
    E = W1.shape[0]
    assert E == N_CORES

    # --- routing (identical fp32 math to the reference) ---
    gvec = np.asarray(GRID, dtype=np.float32)
    u = np.clip((x + np.float32(1.0)) * np.float32(0.5), 0.0, 0.99)
    gi = (u * gvec).astype(np.int32)
    idx = gi[:, 0] + gi[:, 1] * GRID[0] + gi[:, 2] * (GRID[0] * GRID[1])

    order = np.argsort(idx, kind="stable")
    counts = np.bincount(idx, minlength=E)
    starts = np.concatenate([[0], np.cumsum(counts)[:-1]])
    x_sorted = x[order]

    # pairs of 1024 points; the last quad (4 pairs) may be partial
    n_pairs = max(1, int(np.ceil(counts.max() / 1024)))
    nr = (n_pairs + 3) // 4
    cap = nr * PTS_PER_ROUND

    # --- fold the expert-local normalization into layer-1 weights ---
    # xn = s*x + t, s = 2/(emax-emin), t = -2*emin/(emax-emin) - 1
    span = extents_max - extents_min          # [E, 3]
    s = 2.0 / span
    tvec = -2.0 * extents_min / span - 1.0
    # h1_pre = x @ W1e' + b1e',  W1e' = diag(s) @ W1e, b1e' = b1e + t @ W1e
    W1p = W1 * s[:, :, None]                  # [E, 3, H]
    b1p = b1 + np.einsum("ec,ech->eh", tvec, W1)

    in_maps = []
    for e in range(E):
        xe = np.zeros((cap, 3), dtype=np.float32)
        xe[: counts[e]] = x_sorted[starts[e] : starts[e] + counts[e]]
        # xT[32g + 3s + c, r*512 + n] = xe[(4r+g)*1024 + s*512 + n, c]
        xq = (
            xe.reshape(nr, 4, 2, 512, 3)      # r, g, s, n, c
            .transpose(1, 2, 4, 0, 3)         # g, s, c, r, n
            .reshape(4, 6, nr * 512)
            .astype(ml_dtypes.bfloat16)
        )
        xt = xq.reshape(24, nr * 512)
        # S1: w1 block-diag replicated at the 4 row-group offsets
        w1e = W1p[e].astype(ml_dtypes.bfloat16)
        wb_full = np.zeros((128, 384), dtype=ml_dtypes.bfloat16)
        for g in range(4):
            wb_full[32 * g : 32 * g + 3, 0:64] = w1e
            wb_full[32 * g + 3 : 32 * g + 6, 64:128] = w1e
        # w2: [128,128] block-diag of W2
        wb_full[0:64, 128:192] = W2[e].astype(ml_dtypes.bfloat16)
        wb_full[64:128, 192:256] = W2[e].astype(ml_dtypes.bfloat16)
        # w3 block k: slot-s w3 at block col 2k+s
        w3bf = W3[e, :, 0].astype(ml_dtypes.bfloat16)
        for k in range(RG):
            wb_full[0:64, 256 + 16 * k + 2 * k] = w3bf
            wb_full[64:128, 256 + 16 * k + 2 * k + 1] = w3bf
        fb_full = np.stack(
            [np.tile(b1p[e], 2), np.tile(b2[e], 2)], axis=1
        ).astype(np.float32)
        in_maps.append(
            {
                "xT": np.ascontiguousarray(xt),
                "wb": wb_full,
                "fb": fb_full,
            }
        )

    if n_pairs not in _PROGRAM_CACHE:
        _PROGRAM_CACHE[n_pairs] = _build_program(n_pairs)
    nc = _PROGRAM_CACHE[n_pairs]

    res = run_bass_kernel_spmd(nc, in_maps, core_ids=list(range(N_CORES)))
    global LAST_IN_MAPS, LAST_NC, LAST_P
    LAST_RESULTS = res
    LAST_IN_MAPS = in_maps
    LAST_NC = nc
    LAST_P = n_pairs

    # --- unshard: y_dev[p, 2k+s, g*512+n] -> point r*4096+q*512+n; add b3 ---
    n_groups = (nr + RG - 1) // RG
    y_sorted = np.empty(n_pts, dtype=np.float32)
    for e in range(E):
        yd = (res.results[e]["y"]
              .reshape(4, 32, n_groups * 512)[:, 0:16]
              .reshape(4, RG, 2, n_groups, 512))
        ye = (
            yd.transpose(3, 1, 0, 2, 4)       # g, k, p, s, n
            .reshape(n_groups * RG, PTS_PER_ROUND)[:nr]
            .reshape(cap)
            + b3[e, 0]
        )
        y_sorted[starts[e] : starts[e] + counts[e]] = ye[: counts[e]]

    y_full = np.empty(n_pts, dtype=np.float32)
    y_full[order] = y_sorted
    return y_full[:, None]
